# revision 58
# baseline (speedup 1.0000x reference)
"""Distributed single-head causal attention on 8 TRN2 NeuronCores.

Sharding: core = 2*b + h handles batch b and a BALANCED pair of 512-row
query blocks: h=0 -> global t-blocks {0, 3}, h=1 -> {1, 2}. Each t-block i
needs i prefix 512-blocks of K/V, so both pairings cost 3 prefix blocks +
2 diagonal blocks -- no load imbalance.

Per core:
  - x arrives host-transposed/bf16, streamed by TOKEN HALF (all 16 E-slices
    of tokens 0-511, then of 512-1023), so Q/K/V for local block 0 complete
    at the stream midpoint and block-0 attention (diag logits/exps plus its
    K-gather round trip) overlaps the second half of the stream.
  - QKV projection: Q,K packed on 128 PSUM partitions (feature-major, one
    Identity+bias activation per half converts to bf16 -- Identity shares
    the Exp act table so there is no table reload), V in token-major
    orientation (out [128 tok, 64 feat]) so the AV matmuls need no
    transposes. PE warm-up matmuls hold the clock at full speed.
  - One fused K+V AllGather within pairs [[0,1],[2,3],[4,5],[6,7]] (the
    NOCC timing stub emulates it with equivalent per-rank DMA movement);
    V slabs travel WITH their ones-column (denominator trick). K and V
    ride independent DMA chains per (rank, token-half) tile so readers
    wait only on the exact data they need.
  - Attention: 13 exp tiles (4 diag + 9 prefix; [128,1024] double-slot
    where possible). Diagonal masking = multiplicative bf16 triangle
    masks on DVE; cbias -1e30 exp biases kill the prefix tiles past each
    core's causal range, keeping the instruction stream SPMD-uniform.
    Gather layout is rank-major so "global block g" sits at the same
    static address on both cores. ACT (exp) is the critical engine, so
    exps are emitted in data-readiness order (local-K tiles before
    gathered-K tiles).
  - Output is [65, 1024] bf16 (64 feature rows + denominator row); the
    host divides, transposes, and adds the V bias (exact post-softmax).
"""

import os
import sys
import numpy as np

B, T, E, F = 4, 2048, 2048, 64
H = 1024          # q rows per core
NEG = -1e30
KSLAB = F * H          # 65536 bf16 elems: K slab, feature-major [64, 1024]
VSLAB = 128 * 8 * 65   # 66560 bf16 elems: V slab [128, 8*65] incl ones cols
SLAB = KSLAB + VSLAB
# prefix slots: block0 reads gather chunks 0-3; block1 reads 0-3, 8-11, 12-15
B1_CHUNKS = [0, 1, 2, 3, 8, 9, 10, 11, 12, 13, 14, 15]

_cache = {}


def _ensure_path():
    if os.path.isdir("/opt/trn_rl_repo"):
        if "/opt/trn_rl_repo" not in sys.path:
            sys.path.insert(0, "/opt/trn_rl_repo")


def _build():
    _ensure_path()
    import concourse.bass as bass
    import concourse.bacc as bacc
    import concourse.mybir as mybir
    import concourse.tile as tile

    dt = mybir.dt
    AF = mybir.ActivationFunctionType
    f32, bf16 = dt.float32, dt.bfloat16

    nc = bacc.Bacc("TRN2", target_bir_lowering=False, debug=False, num_devices=8)

    xh = nc.dram_tensor("xh", [4096, 512], bf16, kind="ExternalInput")
    wb = nc.dram_tensor("wb", [128, 16 * 192], bf16, kind="ExternalInput")
    cbq = nc.dram_tensor("cbq", [128, 12], f32, kind="ExternalInput")
    out_d = nc.dram_tensor("out", [65, H], bf16, kind="ExternalOutput")

    RG = [[0, 1], [2, 3], [4, 5], [6, 7]]

    with tile.TileContext(nc) as tc:
        with (
            tc.tile_pool(name="const", bufs=1) as constp,
            tc.tile_pool(name="qkv", bufs=1) as qkvp,
            tc.tile_pool(name="sb", bufs=4) as sbp,
            tc.tile_pool(name="dram", bufs=1, space="DRAM") as dram,
        ):
            cb = constp.tile([128, 12], f32, tag="cb")
            # (cb's DMA is issued after the x stream; it isn't needed until
            # the bias adds at ~18us and must not delay the x transfers)
            scr = constp.tile([128, 1], f32, tag="scr")
            # PE warm-up fodder: available almost immediately
            wt = constp.tile([128, 64], bf16, tag="wt")
            nc.vector.memset(wt[:], 1.0)

            # bf16 0/1 triangle masks for the diagonal slot-pairs (kk=0, 1):
            # mask[p, (u, t)] = 1 where t >= 128*(2*kk + u) + p else 0
            dmask = []
            for kk in range(2):
                mk = constp.tile([128, 1024], bf16, tag=f"mk{kk}", name=f"mk{kk}")
                nc.gpsimd.memset(mk[:], 1.0)
                nc.gpsimd.affine_select(
                    out=mk[:], in_=mk[:],
                    compare_op=mybir.AluOpType.is_ge, fill=0.0,
                    base=-128 * 2 * kk, channel_multiplier=-1,
                    pattern=[[-128, 2], [1, 512]],
                )
                dmask.append(mk)

            # per-half q+k tiles (q rows 0-63, k 64-127) + base-0 k copies
            qk_bf = [qkvp.tile([128, 512], bf16, tag=f"qkbf{i}", name=f"qkbf{i}")
                     for i in range(2)]
            k2 = [qkvp.tile([64, 512], bf16, tag=f"k2{i}", name=f"k2{i}")
                  for i in range(2)]
            vmy = [qkvp.tile([128, 4 * 65], bf16, tag=f"vmy{i}", name=f"vmy{i}")
                   for i in range(2)]      # own V + ones cols, per token half
            for i in range(2):
                nc.vector.memset(vmy[i][:], 1.0)  # ones cols, off crit path
            # gathered K (4 chunks per (rank, half) tile) and V(+ones) per rank
            kT_full = [qkvp.tile([64, 512], bf16, tag=f"ktf{n}", name=f"ktf{n}")
                       for n in range(4)]  # n = 2*rank + half
            vg = [qkvp.tile([128, 4 * 65], bf16, tag=f"vg{n}", name=f"vg{n}")
                  for n in range(4)]       # n = 2*rank + half

            kv_in = dram.tile([SLAB], bf16, tag="kvin")
            kv_out = dram.tile([2 * SLAB], bf16, tag="kvout")

            # ---------------- projections, streamed by TOKEN HALF ---------
            # x arrives half-major: all 16 E-slices of tokens 0-511, then of
            # tokens 512-1023. Q/K/V for local block 0 are complete at the
            # stream midpoint, so block-0 attention (diag j0 + b0 prefix,
            # including its K gather round-trip) overlaps the second half of
            # the x stream. PSUM: ps_qk0/1 + ps_v + 2 early-lg [128,1024]
            # tiles = 7 banks inside this scope; the late scope uses 3 lg
            # bufs + 2 po = 8 after these free.
            kvo = kv_out[:].rearrange("(r s) -> r s", r=2)
            dP = [None] * 4
            pP = {}
            with tc.tile_pool(name="xp", bufs=1) as xp, \
                 tc.tile_pool(name="wp", bufs=1) as wp, \
                 tc.tile_pool(name="lge", bufs=2, space="PSUM") as lge, \
                 tc.tile_pool(name="pps", bufs=1, space="PSUM") as pps:
                xbig = xp.tile([128, 16 * H], bf16, tag="xbig")
                wbig = wp.tile([128, 16 * 192], bf16, tag="wbig")
                nc.sync.dma_start(out=wbig[:], in_=wb[:, :])
                nc.sync.dma_start(out=cb[:], in_=cbq[:, :])
                # preload the Exp act table off the critical path
                nc.scalar.activation(scr[:], cb[:, 1:2], AF.Exp)
                for c in range(8):
                    # last chunk of each half splits 3+1 so only e15 waits
                    # on the final bytes' completion semaphore
                    parts = [(0, 4)] if c % 4 != 3 else [(0, 3), (3, 4)]
                    for s0, s1 in parts:
                        nc.sync.dma_start(
                            out=xbig[:, 2048 * c + 512 * s0:2048 * c + 512 * s1],
                            in_=xh[512 * c + 128 * s0:512 * c + 128 * s1, :]
                            .rearrange("(s p) c -> p s c", p=128),
                        )

                ps_qk = [pps.tile([128, 512], f32, tag=f"psqk{i}", name=f"psqk{i}")
                         for i in range(2)]
                ps_v = pps.tile([128, 512], f32, tag="psv")
                # PE warm-up: keep the PE busy with tiny matmuls from ~1.5us
                # so the clock is fully ramped when the first x chunk lands.
                # They scribble on ps_v, which is zeroed afterwards: the 8 V
                # accumulation groups share one PSUM bank and a matmul's
                # start=True clears the WHOLE bank, so V accumulates with
                # start=False onto the memset instead.
                if not os.environ.get("NOWARM"):
                    for _ in range(int(os.environ.get("NWARM", "90"))):
                        nc.tensor.matmul(
                            ps_v[0:64, 0:64], lhsT=wt[:], rhs=wt[:],
                            start=True, stop=True,
                        )
                nc.vector.memset(ps_v[:], 0.0)

                def emit_diag_lg(j, kk):
                    pool = lge if j == 0 else lgp
                    lg = pool.tile([128, 1024], f32, tag="lg", name="lg")
                    for u in range(2):
                        c = 2 * kk + u
                        nc.tensor.matmul(
                            lg[:, 512 * u:512 * (u + 1)],
                            lhsT=k2[j][:, 128 * c:128 * (c + 1)],
                            rhs=qk_bf[j][0:64, :],
                            start=True, stop=True,
                        )
                    p_sb = sbp.tile([128, 1024], bf16, tag="p")
                    nc.scalar.activation(
                        p_sb[:], lg[:], AF.Exp, scale=0.125, bias=cb[:, 1:2],
                    )
                    nc.vector.tensor_mul(p_sb[:], p_sb[:], dmask[kk][:])
                    return p_sb

                def emit_pref_lg(j, chunks, idx):
                    w = 512 * len(chunks)
                    lg = lgp.tile([128, 1024], f32, tag="lg", name="lg")
                    for u, c in enumerate(chunks):
                        nc.tensor.matmul(
                            lg[:, 512 * u:512 * (u + 1)],
                            lhsT=kT_full[c // 4][:, 128 * (c % 4):128 * (c % 4 + 1)],
                            rhs=qk_bf[j][0:64, :],
                            start=True, stop=True,
                        )
                    p_sb = sbp.tile([128, 1024], bf16, tag="p")
                    nc.scalar.activation(
                        p_sb[:, 0:w], lg[:, 0:w], AF.Exp, scale=0.125,
                        bias=cb[:, 2 + idx:3 + idx],
                    )
                    return p_sb

                def proj_e(i, e):
                    we = 192 * e
                    # xbig cols: chunk-major [half][egrp][eslice s][512 tok]
                    xe = 2048 * (4 * i + e // 4) + 512 * (e % 4)
                    nc.tensor.matmul(
                        ps_qk[i][:],
                        lhsT=wbig[:, we:we + 128],
                        rhs=xbig[:, xe:xe + 512],
                        start=(e == 0), stop=(e == 15),
                    )
                    if e == 15:
                        # q+k bias-add on ACT (Identity shares the Exp act
                        # table -> no reload); K ships immediately; the diag
                        # matmuls need k at base partition 0 -> DVE copy.
                        nc.scalar.activation(
                            qk_bf[i][:], ps_qk[i][:], AF.Identity,
                            bias=cb[:, 0:1],
                        )
                        nc.vector.tensor_copy(k2[i][:], qk_bf[i][64:128, :])
                        if os.environ.get("NOCC"):
                            for r in range(2):
                                nc.sync.dma_start(
                                    out=kvo[r:r + 1,
                                            KSLAB // 2 * i:KSLAB // 2 * (i + 1)]
                                    .squeeze(0)
                                    .rearrange("(p c) -> p c", p=64),
                                    in_=qk_bf[i][64:128, :],
                                )
                        else:
                            nc.sync.dma_start(
                                out=kv_in[KSLAB // 2 * i:KSLAB // 2 * (i + 1)],
                                in_=qk_bf[i][64:128, :],
                            )
                    for m in range(4):
                        nc.tensor.matmul(
                            ps_v[:, 256 * i + 64 * m:256 * i + 64 * (m + 1)],
                            lhsT=xbig[:, xe + 128 * m:xe + 128 * (m + 1)],
                            rhs=wbig[:, we + 128:we + 192],
                            start=False, stop=(e == 15),
                        )

                def finish_half(i):
                    # V (+ones) staging and the V-side gather, per half
                    nc.vector.tensor_copy(
                        vmy[i][:].rearrange("p (m c) -> p m c", c=65)[:, :, 0:64],
                        ps_v[:, 256 * i:256 * (i + 1)]
                        .rearrange("p (m c) -> p m c", c=64),
                    )
                    if os.environ.get("NOCC"):
                        for r in range(2):
                            nc.gpsimd.dma_start(
                                out=kvo[r:r + 1,
                                        KSLAB + VSLAB // 2 * i:
                                        KSLAB + VSLAB // 2 * (i + 1)]
                                .squeeze(0).rearrange("(p c) -> p c", p=128),
                                in_=vmy[i][:],
                            )
                    else:
                        nc.gpsimd.dma_start(
                            out=kv_in[KSLAB + VSLAB // 2 * i:
                                      KSLAB + VSLAB // 2 * (i + 1)]
                            .rearrange("(p c) -> p c", p=128),
                            in_=vmy[i][:],
                        )

                def read_back(i):
                    # K and V read-backs for half i (both ranks)
                    for r in range(2):
                        nc.sync.dma_start(
                            out=kT_full[2 * r + i][:],
                            in_=kvo[r:r + 1, KSLAB // 2 * i:KSLAB // 2 * (i + 1)]
                            .squeeze(0).rearrange("(p c) -> p c", p=64),
                        )
                    for r in range(2):
                        nc.gpsimd.dma_start(
                            out=vg[2 * r + i][:],
                            in_=kvo[r:r + 1,
                                    KSLAB + VSLAB // 2 * i:
                                    KSLAB + VSLAB // 2 * (i + 1)]
                            .squeeze(0).rearrange("(p c) -> p c", p=128),
                        )

                # ---- token half 0 ----
                for e in range(16):
                    proj_e(0, e)
                finish_half(0)
                if not os.environ.get("NOCC"):
                    pass  # real collective fires once, after half 1
                else:
                    read_back(0)
                # ---- token half 1 (block-0 diag logits woven in at the
                # points where the PE arrives just as their data lands) ----
                for e in range(16):
                    proj_e(1, e)
                    if e == 3:
                        dP[0] = emit_diag_lg(0, 0)
                    elif e == 5:
                        dP[1] = emit_diag_lg(0, 1)
                finish_half(1)
                if not os.environ.get("NOCC"):
                    nc.gpsimd.collective_compute(
                        "AllGather", mybir.AluOpType.bypass, replica_groups=RG,
                        ins=[kv_in[:].opt()], outs=[kv_out[:].opt()],
                    )
                    read_back(0)
                    read_back(1)
                else:
                    read_back(1)

            # ---------------- late attention (proj PSUM freed) ------------
            with (
                tc.tile_pool(name="lg", bufs=3, space="PSUM") as lgp_,
                tc.tile_pool(name="ot", bufs=1, space="PSUM") as otp,
            ):
                lgp = lgp_
                po = [otp.tile([65, 512], f32, tag=f"po{j}", name=f"po{j}")
                      for j in range(2)]
                av_started = [False, False]

                def emit_av(j, lhsTs, p_sb, stop):
                    for u, lhsT in enumerate(lhsTs):
                        nc.tensor.matmul(
                            po[j][:], lhsT=lhsT,
                            rhs=p_sb[:, 512 * u:512 * (u + 1)],
                            start=(not av_started[j] and u == 0),
                            stop=(stop and u == len(lhsTs) - 1),
                        )
                    av_started[j] = True

                def dvs(j, kk):
                    return [vmy[(4 * j + 2 * kk + u) // 4]
                            [:, 65 * ((4 * j + 2 * kk + u) % 4):
                             65 * ((4 * j + 2 * kk + u) % 4 + 1)]
                            for u in range(2)]

                def gvs(chunks):
                    return [vg[(c // 8) * 2 + (c % 8) // 4]
                            [:, 65 * (c % 4):65 * (c % 4 + 1)]
                            for c in chunks]

                def close_po0():
                    oc0 = sbp.tile([65, 512], bf16, tag="oc0")
                    nc.vector.tensor_copy(oc0[:], po[0][:])
                    nc.sync.dma_start(out=out_d[:, 0:512], in_=oc0[:])

                # exps on ACT (in-order) must run j1 (local data) BEFORE the
                # b0-prefix tiles (whose gathered K lands later)
                dP[2] = emit_diag_lg(1, 0)
                dP[3] = emit_diag_lg(1, 1)
                pP[0] = emit_pref_lg(0, (0, 1), 0)
                pP[1] = emit_pref_lg(0, (2, 3), 1)

                # AV drain queue, in data-readiness order; po0 closes at
                # its b0-prefix AVs, po1 at the last b1 AV
                b1tiles = [(B1_CHUNKS[2 * i], B1_CHUNKS[2 * i + 1])
                           for i in range(5)] + [(14,), (15,)]
                drain = [
                    lambda: emit_av(0, dvs(0, 0), dP[0], False),
                    lambda: emit_av(0, dvs(0, 1), dP[1], False),
                    lambda: emit_av(1, dvs(1, 0), dP[2], False),
                    lambda: emit_av(1, dvs(1, 1), dP[3], False),
                    lambda: emit_av(0, gvs((0, 1)), pP[0], False),
                    lambda: (emit_av(0, gvs((2, 3)), pP[1], True), close_po0()),
                ]
                nd = 0
                pend = []
                for n, chunks in enumerate(b1tiles):
                    p_sb = emit_pref_lg(1, chunks, 2 + n)
                    if nd < len(drain):
                        drain[nd]()
                        nd += 1
                    else:
                        cc, pp = pend.pop(0)
                        emit_av(1, gvs(cc), pp, False)
                    pend.append((chunks, p_sb))
                while nd < len(drain):
                    drain[nd]()
                    nd += 1
                for n, (cc, pp) in enumerate(pend):
                    emit_av(1, gvs(cc), pp, stop=(n == len(pend) - 1))
                oc1 = sbp.tile([65, 512], bf16, tag="oc1")
                nc.vector.tensor_copy(oc1[:], po[1][:])
                nc.sync.dma_start(out=out_d[:, 512:1024], in_=oc1[:])

    nc.compile()
    return nc


def _blocks(h):
    return (0, 3) if h == 0 else (1, 2)


def _in_maps(x, Wq, bq, Wk, bk, Wv, bv):
    import ml_dtypes

    bf16 = ml_dtypes.bfloat16
    wcat = np.concatenate([Wq, Wk, Wv], axis=0).T.astype(np.float32)  # [2048, 192]
    wb = np.ascontiguousarray(
        wcat.reshape(16, 128, 192).transpose(1, 0, 2).reshape(128, 16 * 192)
    ).astype(bf16)
    bqk = np.concatenate([bq, bk]).astype(np.float32)
    maps = []
    for core in range(8):
        b, h = core // 2, core % 2
        gA, gB = _blocks(h)
        xr = np.concatenate(
            [x[b, 512 * gA:512 * (gA + 1), :], x[b, 512 * gB:512 * (gB + 1), :]],
            axis=0,
        )  # [1024, 2048] local token order
        xT = xr.T.astype(np.float32)  # [2048, 1024] E-major
        # DMA layout: 8 chunks (half-major) x [4 e-slices, 128 p, 512 tok]:
        # chunk c covers token half c//4 and e-slices 4*(c%4)..4*(c%4)+3
        xh = np.ascontiguousarray(
            xT.reshape(4, 4, 128, 2, 512)      # [egrp, esub, p, half, tok]
            .transpose(3, 0, 1, 2, 4)          # [half, egrp, esub, p, tok]
            .reshape(4096, 512)
        ).astype(bf16)
        cbq = np.zeros((128, 12), np.float32)
        cbq[:, 0] = bqk
        # prefix exp-tile biases (cols 2-10): tiles = [b0 (0,1),(2,3);
        # b1 (0,1),(2,3),(8,9),(10,11),(12,13),(14),(15)]
        # h=0: block0 (global 0) has no prefix -> cols 2,3 = NEG
        # h=1: block1 (global 2) doesn't need chunks 12-15 -> cols 8,9,10 = NEG
        if h == 0:
            cbq[:, 2] = NEG
            cbq[:, 3] = NEG
        else:
            cbq[:, 8] = NEG
            cbq[:, 9] = NEG
            cbq[:, 10] = NEG
        maps.append({"xh": xh, "wb": wb, "cbq": cbq})
    return maps


def kernel(x, Wq, bq, Wk, bk, Wv, bv):
    _ensure_path()
    from concourse.bass_utils import run_bass_kernel_spmd

    if "nc" not in _cache:
        _cache["nc"] = _build()
    nc = _cache["nc"]
    maps = _in_maps(x, Wq, bq, Wk, bk, Wv, bv)
    res = run_bass_kernel_spmd(nc, maps, core_ids=list(range(8)),
                               trace=bool(int(os.environ.get("KTRACE", "0"))))
    _cache["last"] = res
    out = np.empty((B, T, F), np.float32)
    for core in range(8):
        b, h = core // 2, core % 2
        r = res.results[core]["out"].astype(np.float32)  # [65, 1024] bf16
        o = (r[0:64, :] / r[64:65, :]).T + bv[None, :]  # [1024, 64]
        for j, g in enumerate(_blocks(h)):
            out[b, 512 * g:512 * (g + 1), :] = o[512 * j:512 * (j + 1), :]
    return out


# revision 59
# speedup vs baseline: 1.0061x; 1.0061x over previous
"""Distributed single-head causal attention on 8 TRN2 NeuronCores.

Sharding: core = 2*b + h handles batch b and a BALANCED pair of 512-row
query blocks: h=0 -> global t-blocks {0, 3}, h=1 -> {1, 2}. Each t-block i
needs i prefix 512-blocks of K/V, so both pairings cost 3 prefix blocks +
2 diagonal blocks -- no load imbalance.

Per core:
  - x arrives host-transposed/bf16, streamed by TOKEN HALF (all 16 E-slices
    of tokens 0-511, then of 512-1023), so Q/K/V for local block 0 complete
    at the stream midpoint and block-0 attention (diag logits/exps plus its
    K-gather round trip) overlaps the second half of the stream.
  - QKV projection: Q,K packed on 128 PSUM partitions (feature-major, one
    Identity+bias activation per half converts to bf16 -- Identity shares
    the Exp act table so there is no table reload), V in token-major
    orientation (out [128 tok, 64 feat]) so the AV matmuls need no
    transposes. PE warm-up matmuls hold the clock at full speed.
  - One fused K+V AllGather within pairs [[0,1],[2,3],[4,5],[6,7]] (the
    NOCC timing stub emulates it with equivalent per-rank DMA movement);
    V slabs travel WITH their ones-column (denominator trick). K and V
    ride independent DMA chains per (rank, token-half) tile so readers
    wait only on the exact data they need.
  - Attention: 13 exp tiles (4 diag + 9 prefix; [128,1024] double-slot
    where possible). Diagonal masking = multiplicative bf16 triangle
    masks on DVE; cbias -1e30 exp biases kill the prefix tiles past each
    core's causal range, keeping the instruction stream SPMD-uniform.
    Gather layout is rank-major so "global block g" sits at the same
    static address on both cores. ACT (exp) is the critical engine, so
    exps are emitted in data-readiness order (local-K tiles before
    gathered-K tiles).
  - Output is [65, 1024] bf16 (64 feature rows + denominator row); the
    host divides, transposes, and adds the V bias (exact post-softmax).
"""

import os
import sys
import numpy as np

B, T, E, F = 4, 2048, 2048, 64
H = 1024          # q rows per core
NEG = -1e30
KSLAB = F * H          # 65536 bf16 elems: K slab, feature-major [64, 1024]
VSLAB = 128 * 8 * 65   # 66560 bf16 elems: V slab [128, 8*65] incl ones cols
SLAB = KSLAB + VSLAB
# prefix slots: block0 reads gather chunks 0-3; block1 reads 0-3, 8-11, 12-15
B1_CHUNKS = [0, 1, 2, 3, 8, 9, 10, 11, 12, 13, 14, 15]

_cache = {}


def _ensure_path():
    if os.path.isdir("/opt/trn_rl_repo"):
        if "/opt/trn_rl_repo" not in sys.path:
            sys.path.insert(0, "/opt/trn_rl_repo")


def _build():
    _ensure_path()
    import concourse.bass as bass
    import concourse.bacc as bacc
    import concourse.mybir as mybir
    import concourse.tile as tile

    dt = mybir.dt
    AF = mybir.ActivationFunctionType
    f32, bf16 = dt.float32, dt.bfloat16

    nc = bacc.Bacc("TRN2", target_bir_lowering=False, debug=False, num_devices=8)

    xh = nc.dram_tensor("xh", [4096, 512], bf16, kind="ExternalInput")
    wb = nc.dram_tensor("wb", [128, 16 * 192], bf16, kind="ExternalInput")
    cbq = nc.dram_tensor("cbq", [128, 12], f32, kind="ExternalInput")
    out_d = nc.dram_tensor("out", [65, H], bf16, kind="ExternalOutput")

    RG = [[0, 1], [2, 3], [4, 5], [6, 7]]

    with tile.TileContext(nc) as tc:
        with (
            tc.tile_pool(name="const", bufs=1) as constp,
            tc.tile_pool(name="qkv", bufs=1) as qkvp,
            tc.tile_pool(name="sb", bufs=4) as sbp,
            tc.tile_pool(name="dram", bufs=1, space="DRAM") as dram,
        ):
            cb = constp.tile([128, 12], f32, tag="cb")
            # (cb's DMA is issued after the x stream; it isn't needed until
            # the bias adds at ~18us and must not delay the x transfers)
            scr = constp.tile([128, 1], f32, tag="scr")
            # PE warm-up fodder: available almost immediately
            wt = constp.tile([128, 64], bf16, tag="wt")
            nc.vector.memset(wt[:], 1.0)

            # bf16 0/1 triangle masks for the diagonal slot-pairs (kk=0, 1):
            # mask[p, (u, t)] = 1 where t >= 128*(2*kk + u) + p else 0
            dmask = []
            for kk in range(2):
                mk = constp.tile([128, 1024], bf16, tag=f"mk{kk}", name=f"mk{kk}")
                nc.gpsimd.memset(mk[:], 1.0)
                nc.gpsimd.affine_select(
                    out=mk[:], in_=mk[:],
                    compare_op=mybir.AluOpType.is_ge, fill=0.0,
                    base=-128 * 2 * kk, channel_multiplier=-1,
                    pattern=[[-128, 2], [1, 512]],
                )
                dmask.append(mk)

            # per-half q+k tiles (q rows 0-63, k 64-127) + base-0 k copies
            qk_bf = [qkvp.tile([128, 512], bf16, tag=f"qkbf{i}", name=f"qkbf{i}")
                     for i in range(2)]
            k2 = [qkvp.tile([64, 512], bf16, tag=f"k2{i}", name=f"k2{i}")
                  for i in range(2)]
            vmy = [qkvp.tile([128, 4 * 65], bf16, tag=f"vmy{i}", name=f"vmy{i}")
                   for i in range(2)]      # own V + ones cols, per token half
            for i in range(2):
                nc.vector.memset(vmy[i][:], 1.0)  # ones cols, off crit path
            # gathered K (4 chunks per (rank, half) tile) and V(+ones) per rank
            kT_full = [qkvp.tile([64, 512], bf16, tag=f"ktf{n}", name=f"ktf{n}")
                       for n in range(4)]  # n = 2*rank + half
            vg = [qkvp.tile([128, 4 * 65], bf16, tag=f"vg{n}", name=f"vg{n}")
                  for n in range(4)]       # n = 2*rank + half

            kv_in = dram.tile([SLAB], bf16, tag="kvin")
            kv_out = dram.tile([2 * SLAB], bf16, tag="kvout")

            # ---------------- projections, streamed by TOKEN HALF ---------
            # x arrives half-major: all 16 E-slices of tokens 0-511, then of
            # tokens 512-1023. Q/K/V for local block 0 are complete at the
            # stream midpoint, so block-0 attention (diag j0 + b0 prefix,
            # including its K gather round-trip) overlaps the second half of
            # the x stream. PSUM: ps_qk0/1 + ps_v + 2 early-lg [128,1024]
            # tiles = 7 banks inside this scope; the late scope uses 3 lg
            # bufs + 2 po = 8 after these free.
            kvo = kv_out[:].rearrange("(r s) -> r s", r=2)
            dP = [None] * 4
            pP = {}
            with tc.tile_pool(name="xp", bufs=1) as xp, \
                 tc.tile_pool(name="wp", bufs=1) as wp, \
                 tc.tile_pool(name="lge", bufs=2, space="PSUM") as lge, \
                 tc.tile_pool(name="pps", bufs=1, space="PSUM") as pps:
                xbig = xp.tile([128, 16 * H], bf16, tag="xbig")
                wbig = wp.tile([128, 16 * 192], bf16, tag="wbig")
                nc.sync.dma_start(out=wbig[:], in_=wb[:, :])
                nc.sync.dma_start(out=cb[:], in_=cbq[:, :])
                # preload the Exp act table off the critical path
                nc.scalar.activation(scr[:], cb[:, 1:2], AF.Exp)
                def x_dma(c, s0, s1):
                    nc.sync.dma_start(
                        out=xbig[:, 2048 * c + 512 * s0:2048 * c + 512 * s1],
                        in_=xh[512 * c + 128 * s0:512 * c + 128 * s1, :]
                        .rearrange("(s p) c -> p s c", p=128),
                    )
                # last chunk of each half splits 3+1 so only e15 waits on the
                # final bytes' semaphore; the very last sub-chunk is emitted
                # after half 0 so the K-i0 gather writes queue ahead of it
                # on the serial DMA device
                for c in range(7):
                    if c % 4 != 3:
                        x_dma(c, 0, 4)
                    else:
                        x_dma(c, 0, 3)
                        x_dma(c, 3, 4)
                x_dma(7, 0, 3)

                ps_qk = [pps.tile([128, 512], f32, tag=f"psqk{i}", name=f"psqk{i}")
                         for i in range(2)]
                ps_v = pps.tile([128, 512], f32, tag="psv")
                # PE warm-up: keep the PE busy with tiny matmuls from ~1.5us
                # so the clock is fully ramped when the first x chunk lands.
                # They scribble on ps_v, which is zeroed afterwards: the 8 V
                # accumulation groups share one PSUM bank and a matmul's
                # start=True clears the WHOLE bank, so V accumulates with
                # start=False onto the memset instead.
                if not os.environ.get("NOWARM"):
                    for _ in range(int(os.environ.get("NWARM", "90"))):
                        nc.tensor.matmul(
                            ps_v[0:64, 0:64], lhsT=wt[:], rhs=wt[:],
                            start=True, stop=True,
                        )
                nc.vector.memset(ps_v[:], 0.0)

                def emit_diag_lg(j, kk):
                    pool = lge if j == 0 else lgp
                    lg = pool.tile([128, 1024], f32, tag="lg", name="lg")
                    for u in range(2):
                        c = 2 * kk + u
                        nc.tensor.matmul(
                            lg[:, 512 * u:512 * (u + 1)],
                            lhsT=k2[j][:, 128 * c:128 * (c + 1)],
                            rhs=qk_bf[j][0:64, :],
                            start=True, stop=True,
                        )
                    p_sb = sbp.tile([128, 1024], bf16, tag="p")
                    nc.scalar.activation(
                        p_sb[:], lg[:], AF.Exp, scale=0.125, bias=cb[:, 1:2],
                    )
                    nc.vector.tensor_mul(p_sb[:], p_sb[:], dmask[kk][:])
                    return p_sb

                def emit_pref_lg(j, chunks, idx):
                    w = 512 * len(chunks)
                    lg = lgp.tile([128, 1024], f32, tag="lg", name="lg")
                    for u, c in enumerate(chunks):
                        nc.tensor.matmul(
                            lg[:, 512 * u:512 * (u + 1)],
                            lhsT=kT_full[c // 4][:, 128 * (c % 4):128 * (c % 4 + 1)],
                            rhs=qk_bf[j][0:64, :],
                            start=True, stop=True,
                        )
                    p_sb = sbp.tile([128, 1024], bf16, tag="p")
                    nc.scalar.activation(
                        p_sb[:, 0:w], lg[:, 0:w], AF.Exp, scale=0.125,
                        bias=cb[:, 2 + idx:3 + idx],
                    )
                    return p_sb

                def proj_e(i, e):
                    we = 192 * e
                    # xbig cols: chunk-major [half][egrp][eslice s][512 tok]
                    xe = 2048 * (4 * i + e // 4) + 512 * (e % 4)
                    nc.tensor.matmul(
                        ps_qk[i][:],
                        lhsT=wbig[:, we:we + 128],
                        rhs=xbig[:, xe:xe + 512],
                        start=(e == 0), stop=(e == 15),
                    )
                    if e == 15:
                        # q+k bias-add on ACT (Identity shares the Exp act
                        # table -> no reload); K ships immediately; the diag
                        # matmuls need k at base partition 0 -> DVE copy.
                        nc.scalar.activation(
                            qk_bf[i][:], ps_qk[i][:], AF.Identity,
                            bias=cb[:, 0:1],
                        )
                        nc.vector.tensor_copy(k2[i][:], qk_bf[i][64:128, :])
                        if os.environ.get("NOCC"):
                            for r in range(2):
                                nc.sync.dma_start(
                                    out=kvo[r:r + 1,
                                            KSLAB // 2 * i:KSLAB // 2 * (i + 1)]
                                    .squeeze(0)
                                    .rearrange("(p c) -> p c", p=64),
                                    in_=qk_bf[i][64:128, :],
                                )
                        else:
                            nc.sync.dma_start(
                                out=kv_in[KSLAB // 2 * i:KSLAB // 2 * (i + 1)],
                                in_=qk_bf[i][64:128, :],
                            )
                    for m in range(4):
                        nc.tensor.matmul(
                            ps_v[:, 256 * i + 64 * m:256 * i + 64 * (m + 1)],
                            lhsT=xbig[:, xe + 128 * m:xe + 128 * (m + 1)],
                            rhs=wbig[:, we + 128:we + 192],
                            start=False, stop=(e == 15),
                        )

                def finish_half(i):
                    # V (+ones) staging and the V-side gather, per half
                    nc.vector.tensor_copy(
                        vmy[i][:].rearrange("p (m c) -> p m c", c=65)[:, :, 0:64],
                        ps_v[:, 256 * i:256 * (i + 1)]
                        .rearrange("p (m c) -> p m c", c=64),
                    )
                    if os.environ.get("NOCC"):
                        for r in range(2):
                            nc.gpsimd.dma_start(
                                out=kvo[r:r + 1,
                                        KSLAB + VSLAB // 2 * i:
                                        KSLAB + VSLAB // 2 * (i + 1)]
                                .squeeze(0).rearrange("(p c) -> p c", p=128),
                                in_=vmy[i][:],
                            )
                    else:
                        nc.gpsimd.dma_start(
                            out=kv_in[KSLAB + VSLAB // 2 * i:
                                      KSLAB + VSLAB // 2 * (i + 1)]
                            .rearrange("(p c) -> p c", p=128),
                            in_=vmy[i][:],
                        )

                def read_back(i):
                    # K and V read-backs for half i (both ranks)
                    for r in range(2):
                        nc.sync.dma_start(
                            out=kT_full[2 * r + i][:],
                            in_=kvo[r:r + 1, KSLAB // 2 * i:KSLAB // 2 * (i + 1)]
                            .squeeze(0).rearrange("(p c) -> p c", p=64),
                        )
                    for r in range(2):
                        nc.gpsimd.dma_start(
                            out=vg[2 * r + i][:],
                            in_=kvo[r:r + 1,
                                    KSLAB + VSLAB // 2 * i:
                                    KSLAB + VSLAB // 2 * (i + 1)]
                            .squeeze(0).rearrange("(p c) -> p c", p=128),
                        )

                # ---- token half 0 ----
                for e in range(16):
                    proj_e(0, e)
                x_dma(7, 3, 4)
                finish_half(0)
                if not os.environ.get("NOCC"):
                    pass  # real collective fires once, after half 1
                else:
                    read_back(0)
                # ---- token half 1 (block-0 diag logits woven in at the
                # points where the PE arrives just as their data lands) ----
                for e in range(16):
                    proj_e(1, e)
                    if e == 3:
                        dP[0] = emit_diag_lg(0, 0)
                    elif e == 5:
                        dP[1] = emit_diag_lg(0, 1)
                finish_half(1)
                if not os.environ.get("NOCC"):
                    nc.gpsimd.collective_compute(
                        "AllGather", mybir.AluOpType.bypass, replica_groups=RG,
                        ins=[kv_in[:].opt()], outs=[kv_out[:].opt()],
                    )
                    read_back(0)
                    read_back(1)
                else:
                    read_back(1)

            # ---------------- late attention (proj PSUM freed) ------------
            with (
                tc.tile_pool(name="lg", bufs=3, space="PSUM") as lgp_,
                tc.tile_pool(name="ot", bufs=1, space="PSUM") as otp,
            ):
                lgp = lgp_
                po = [otp.tile([65, 512], f32, tag=f"po{j}", name=f"po{j}")
                      for j in range(2)]
                av_started = [False, False]

                def emit_av(j, lhsTs, p_sb, stop):
                    for u, lhsT in enumerate(lhsTs):
                        nc.tensor.matmul(
                            po[j][:], lhsT=lhsT,
                            rhs=p_sb[:, 512 * u:512 * (u + 1)],
                            start=(not av_started[j] and u == 0),
                            stop=(stop and u == len(lhsTs) - 1),
                        )
                    av_started[j] = True

                def dvs(j, kk):
                    return [vmy[(4 * j + 2 * kk + u) // 4]
                            [:, 65 * ((4 * j + 2 * kk + u) % 4):
                             65 * ((4 * j + 2 * kk + u) % 4 + 1)]
                            for u in range(2)]

                def gvs(chunks):
                    return [vg[(c // 8) * 2 + (c % 8) // 4]
                            [:, 65 * (c % 4):65 * (c % 4 + 1)]
                            for c in chunks]

                def close_po0():
                    oc0 = sbp.tile([65, 512], bf16, tag="oc0")
                    nc.vector.tensor_copy(oc0[:], po[0][:])
                    nc.sync.dma_start(out=out_d[:, 0:512], in_=oc0[:])

                # exps on ACT (in-order) must run j1 (local data) BEFORE the
                # b0-prefix tiles (whose gathered K lands later)
                pP[0] = emit_pref_lg(0, (0, 1), 0)
                pP[1] = emit_pref_lg(0, (2, 3), 1)
                dP[2] = emit_diag_lg(1, 0)
                dP[3] = emit_diag_lg(1, 1)

                # AV drain queue, in data-readiness order; po0 closes at
                # its b0-prefix AVs, po1 at the last b1 AV
                b1tiles = [(B1_CHUNKS[2 * i], B1_CHUNKS[2 * i + 1])
                           for i in range(5)] + [(14,), (15,)]
                drain = [
                    lambda: emit_av(0, dvs(0, 0), dP[0], False),
                    lambda: emit_av(0, dvs(0, 1), dP[1], False),
                    lambda: emit_av(1, dvs(1, 0), dP[2], False),
                    lambda: emit_av(1, dvs(1, 1), dP[3], False),
                    lambda: emit_av(0, gvs((0, 1)), pP[0], False),
                    lambda: (emit_av(0, gvs((2, 3)), pP[1], True), close_po0()),
                ]
                nd = 0
                pend = []
                for n, chunks in enumerate(b1tiles):
                    p_sb = emit_pref_lg(1, chunks, 2 + n)
                    if nd < len(drain):
                        drain[nd]()
                        nd += 1
                    else:
                        cc, pp = pend.pop(0)
                        emit_av(1, gvs(cc), pp, False)
                    pend.append((chunks, p_sb))
                while nd < len(drain):
                    drain[nd]()
                    nd += 1
                for n, (cc, pp) in enumerate(pend):
                    emit_av(1, gvs(cc), pp, stop=(n == len(pend) - 1))
                oc1 = sbp.tile([65, 512], bf16, tag="oc1")
                nc.vector.tensor_copy(oc1[:], po[1][:])
                nc.sync.dma_start(out=out_d[:, 512:1024], in_=oc1[:])

    nc.compile()
    return nc


def _blocks(h):
    return (0, 3) if h == 0 else (1, 2)


def _in_maps(x, Wq, bq, Wk, bk, Wv, bv):
    import ml_dtypes

    bf16 = ml_dtypes.bfloat16
    wcat = np.concatenate([Wq, Wk, Wv], axis=0).T.astype(np.float32)  # [2048, 192]
    wb = np.ascontiguousarray(
        wcat.reshape(16, 128, 192).transpose(1, 0, 2).reshape(128, 16 * 192)
    ).astype(bf16)
    bqk = np.concatenate([bq, bk]).astype(np.float32)
    maps = []
    for core in range(8):
        b, h = core // 2, core % 2
        gA, gB = _blocks(h)
        xr = np.concatenate(
            [x[b, 512 * gA:512 * (gA + 1), :], x[b, 512 * gB:512 * (gB + 1), :]],
            axis=0,
        )  # [1024, 2048] local token order
        xT = xr.T.astype(np.float32)  # [2048, 1024] E-major
        # DMA layout: 8 chunks (half-major) x [4 e-slices, 128 p, 512 tok]:
        # chunk c covers token half c//4 and e-slices 4*(c%4)..4*(c%4)+3
        xh = np.ascontiguousarray(
            xT.reshape(4, 4, 128, 2, 512)      # [egrp, esub, p, half, tok]
            .transpose(3, 0, 1, 2, 4)          # [half, egrp, esub, p, tok]
            .reshape(4096, 512)
        ).astype(bf16)
        cbq = np.zeros((128, 12), np.float32)
        cbq[:, 0] = bqk
        # prefix exp-tile biases (cols 2-10): tiles = [b0 (0,1),(2,3);
        # b1 (0,1),(2,3),(8,9),(10,11),(12,13),(14),(15)]
        # h=0: block0 (global 0) has no prefix -> cols 2,3 = NEG
        # h=1: block1 (global 2) doesn't need chunks 12-15 -> cols 8,9,10 = NEG
        if h == 0:
            cbq[:, 2] = NEG
            cbq[:, 3] = NEG
        else:
            cbq[:, 8] = NEG
            cbq[:, 9] = NEG
            cbq[:, 10] = NEG
        maps.append({"xh": xh, "wb": wb, "cbq": cbq})
    return maps


def kernel(x, Wq, bq, Wk, bk, Wv, bv):
    _ensure_path()
    from concourse.bass_utils import run_bass_kernel_spmd

    if "nc" not in _cache:
        _cache["nc"] = _build()
    nc = _cache["nc"]
    maps = _in_maps(x, Wq, bq, Wk, bk, Wv, bv)
    res = run_bass_kernel_spmd(nc, maps, core_ids=list(range(8)),
                               trace=bool(int(os.environ.get("KTRACE", "0"))))
    _cache["last"] = res
    out = np.empty((B, T, F), np.float32)
    for core in range(8):
        b, h = core // 2, core % 2
        r = res.results[core]["out"].astype(np.float32)  # [65, 1024] bf16
        o = (r[0:64, :] / r[64:65, :]).T + bv[None, :]  # [1024, 64]
        for j, g in enumerate(_blocks(h)):
            out[b, 512 * g:512 * (g + 1), :] = o[512 * j:512 * (j + 1), :]
    return out
